# revision 5
# baseline (speedup 1.0000x reference)
"""Trainium2 Bass kernel for a 16-head causal MHA layer with relative-position
bias (B=2, S=2048, D=1024, H=16, HD=64), distributed over 8 NeuronCores.

Sharding: tensor parallel over heads — core c computes heads {2c, 2c+1} for
both batches.  The output projection is sharded over its input dim, so each
core returns a partial (B, S, D) output; the partials are summed on the host
(plus proj_b).

Per-core device pipeline (matmul operands in fp16; single-shot matmuls write
fp16 PSUM so evictions run in the DVE 2x perf mode):
  1. QKV projections in transposed layout: QT/KT/VT (128=2*HD, 2048) from
     XT (D, S) chunks x weight-slice chunks.  K' = K + rpr[positions] folded
     into the same PSUM accumulation group via a stacked-identity matmul
     (rank-64 row-tiling update), so the eviction is a plain copy.
  2. Attention per head in S^T layout: scores S^T(j,i) = K'^T-slice . Q-slice
     (both heads packed in one PE pass via tile_position row tiling, which
     runs concurrently on HW), exp on the scalar engine (scale folded in),
     causal masking via a 0/1-tril multiply on the exp OUTPUT (gpsimd),
     then OT_aug(65, i) += V_aug(j, 65)^T . P^T with a ones-column producing
     the softmax denominators for free.  Fully-masked j-blocks are skipped,
     fully-masked lead columns of diagonal blocks are not computed.
  3. Normalize: OT = OT_aug[0:64] * bcast(1/OT_aug[64]) (bcast via K=1 PE
     matmul; multiply reads both PSUM operands directly).
  4. Output projection: y_partial(s, e) += OT2^T . pwT -> fp16 PSUM -> 2x
     eviction into a 4-block staging tile, one DMA per (b, it).
"""

import sys

import numpy as np

try:
    import concourse.bass as bass  # noqa: F401
except ImportError:
    sys.path.insert(0, "/opt/trn_rl_repo")

import concourse.bass as bass
import concourse.mybir as mybir
import concourse.tile as tile
from concourse import bacc
from concourse.bass_utils import run_bass_kernel_spmd

B, S, D, H = 2, 2048, 1024, 16
HD = D // H  # 64
SCALE = HD**-0.5
N_CORES = 8
HPC = H // N_CORES  # heads per core = 2
DL = HPC * HD  # local head channels = 128
NJ = S // 128  # 16 j-chunks of 128
NI = S // 512  # 4 i-blocks of 512
KC = D // 128  # 8 contraction chunks of 128

F32 = mybir.dt.float32
F32R = mybir.dt.float32r
F16 = mybir.dt.float16

import os

# tunables
TRIL_ENGINE = os.environ.get("K_TRIL", "vector")  # causal 0/1 multiply engine
Y_TO_ACT = int(os.environ.get("K_YACT", "1"))  # y evictions to scalar, of 4
AV_SPLIT = bool(int(os.environ.get("K_AVSPLIT", "1")))  # defer masked AV
SC_TRIM = True  # skip fully-masked lead columns in the scores matmul
NORM_BC = os.environ.get("K_NORM", "gpsimd")  # "gpsimd" | "pe"
SC_SPLIT = bool(int(os.environ.get("K_SCSPLIT", "0")))  # per-head score tiles
PT_BUFS = int(os.environ.get("K_PTBUFS", "3"))  # exp-output ring depth
# (either way the multiply reads only one PSUM operand — a HW requirement)
SKIP_XT = bool(int(os.environ.get("K_SKIP_XT", "0")))  # ablation: no x loads
SKIP_Y = bool(int(os.environ.get("K_SKIP_Y", "0")))  # ablation: no y stores
HALF_EXP = bool(int(os.environ.get("K_HALF_EXP", "0")))  # ablation probe
HALF_AV = bool(int(os.environ.get("K_HALF_AV", "0")))  # ablation probe
NO_NORM = bool(int(os.environ.get("K_NO_NORM", "0")))  # ablation probe
NO_YEV = bool(int(os.environ.get("K_NO_YEV", "0")))  # ablation probe
NOREC = bool(int(os.environ.get("K_NOREC", "0")))  # ablation probe
NORM_REC = os.environ.get("K_REC", "act")  # "act": 1/x = exp(-ln x); "dve"
NO_ATT = bool(int(os.environ.get("K_NO_ATT", "0")))  # ablation probe
NO_PROJ = bool(int(os.environ.get("K_NO_PROJ", "0")))  # ablation probe

_BUILD_CACHE: dict = {}


def _emit(nc, tc, t, mode, niter):
    xt = t["xt"].ap()  # (B, 128, 8*2048) f16 (see _prep_inputs layout)
    wT = [t["wqT"].ap(), t["wkT"].ap(), t["wvT"].ap()]  # (D, DL) f16
    pwT = t["pwT"].ap()  # (DL, D) f16
    rpr2 = t["rpr2T"].ap()  # (128, S) f16 : b-th 64 rows = rprT for batch b
    tril01 = t["tril01"].ap()  # (128, 128) f16: 1 on/below diag else 0
    ident = t["ident"].ap()  # (128, 128) f16
    i2 = t["i2"].ap()  # (128, 128) f16: both 64-row halves are [I64 | I64]
    onesc = t["onesc"].ap()  # (128, 1) f16
    ones1 = t["ones1"].ap()  # (1, HD) f16
    y = t["y"].ap()  # (B, NI, 128, 4*1024) f16
    maskT = t["maskT"].ap() if "maskT" in t else None  # (S, S) f32

    # Pre-load the natural_log_exp_and_others act-function set (id 6): it
    # serves Exp, Ln AND Copy, so the compiler's table-load pass finds every
    # activation already satisfiable and inserts no per-use reloads (the
    # greedy per-func choice would otherwise toggle exp_and_others <->
    # natural_log around each Ln pair at 1.28us per reload).
    nc.scalar.add_instruction(mybir.InstLoadActFuncSet(
        act_func_set_id=6, name=nc.get_next_instruction_name(),
        engine=mybir.EngineType.Activation, ins=[], outs=[]))

    ctxs = [
        tc.tile_pool(name="consts", bufs=1),
        tc.tile_pool(name="xt", bufs=1),
        tc.tile_pool(name="qkv", bufs=1),
        tc.tile_pool(name="va", bufs=1),
        tc.tile_pool(name="pt", bufs=PT_BUFS),
        tc.tile_pool(name="sm", bufs=2),
        tc.tile_pool(name="ysb", bufs=2),
        tc.tile_pool(name="ps_mm", bufs=2, space="PSUM"),
        tc.tile_pool(name="ps_sc", bufs=4 if SC_SPLIT else 2, space="PSUM"),
        tc.tile_pool(name="ps_ot", bufs=2, space="PSUM"),
    ]
    if maskT is not None:
        ctxs.append(tc.tile_pool(name="mk", bufs=4))
    pools = [c.__enter__() for c in ctxs]
    (consts, xtp, qkvp, vap, ptp, smp, ysbp, ps_mm, ps_sc, ps_ot) = pools[:10]
    mkp = pools[10] if maskT is not None else None

    # --- persistent constants (loaded once, outside the timing loop) ---
    w_t = [[consts.tile([128, DL], F16, tag=f"w{p}{k}", name=f"w{p}{k}")
            for k in range(KC)] for p in range(3)]
    for p in range(3):
        for k in range(KC):
            nc.sync.dma_start(w_t[p][k][:], wT[p][k * 128:(k + 1) * 128, :])
    pw_t = consts.tile([DL, D], F16, tag="pw")
    nc.sync.dma_start(pw_t[:], pwT)
    rpr_t = consts.tile([128, S], F16, tag="rpr")
    nc.sync.dma_start(rpr_t[:], rpr2)
    tril_t = consts.tile([128, 128], F16, tag="tril")
    nc.sync.dma_start(tril_t[:], tril01)
    id_t = consts.tile([128, 128], F16, tag="id")
    nc.sync.dma_start(id_t[:], ident)
    i2_t = consts.tile([128, 128], F16, tag="i2")
    nc.sync.dma_start(i2_t[:], i2)
    ones_t = consts.tile([128, 1], F16, tag="ones")
    nc.sync.dma_start(ones_t[:], onesc)
    ones1_t = consts.tile([1, HD], F16, tag="ones1")
    nc.sync.dma_start(ones1_t[:], ones1)

    # persistent V_aug tiles with the ones columns preset once:
    # layout [v_h0(0:64) | 1(64) | v_h1(65:129) | 1(129)]
    va_t = {(b, j): vap.tile([128, 2 * (HD + 1)], F16, tag=f"va{b}_{j}",
                             name=f"va{b}_{j}")
            for b in range(B) for j in range(NJ)}
    for (b, j), va in va_t.items():
        nc.vector.tensor_copy(va[:, HD:HD + 1], ones_t[:])
        nc.vector.tensor_copy(va[:, 2 * HD + 1:2 * HD + 2], ones_t[:])

    # persistent tiles referenced across loop iterations (bufs=1 tags)
    xts = {b: xtp.tile([128, KC * S], F16, tag=f"xt{b}", name=f"xt{b}")
           for b in range(B)}
    qkv = {b: (qkvp.tile([128, S], F16, tag=f"qt{b}", name=f"qt{b}"),
               qkvp.tile([128, S], F16, tag=f"kt{b}", name=f"kt{b}"),
               qkvp.tile([128, S], F16, tag=f"vt{b}", name=f"vt{b}"))
           for b in range(B)}
    ot2s = {b: qkvp.tile([128, S], F16, tag=f"ot2_{b}", name=f"ot2_{b}")
            for b in range(B)}

    def make_body():

        def emit_loads(b):
            if not SKIP_XT:
                # per-k-chunk DMAs so the first QKV matmul (which reads only
                # chunk 0) can start ~1/8 of the way into the transfer
                for k in range(KC):
                    nc.sync.dma_start(xts[b][:, k * S:(k + 1) * S],
                                      xt[b, :, k * S:(k + 1) * S])

        def emit_qkv_group(b, g):
            # g in 0..11: projection p = g // NI, 512-col block sb = g % NI
            p, sb = g // NI, g % NI
            dst = qkv[b][p]
            ps = ps_mm.tile([128, 512], F32, tag="mm")
            for k in range(KC):
                nc.tensor.matmul(
                    ps[:], w_t[p][k][:],
                    xts[b][:, k * S + sb * 512:k * S + (sb + 1) * 512],
                    start=(k == 0), stop=(p != 1 and k == KC - 1))
            sl = slice(sb * 512, (sb + 1) * 512)
            if p == 1:
                # K' = K + rpr via a rank-64 stacked-identity matmul in the
                # same accumulation group (same bias for both head halves)
                nc.tensor.matmul(ps[:], i2_t[64 * b:64 * b + 64, :],
                                 rpr_t[64 * b:64 * b + 64, sl],
                                 start=False, stop=True)
            nc.vector.tensor_copy(dst[:, sl], ps[:])

        def emit_va(b):
            vt = qkv[b][2]
            for j in range(NJ):
                tp = ps_mm.tile([128, 128], F16, tag="mm", name="tp")
                nc.tensor.transpose(tp[:], vt[:, j * 128:(j + 1) * 128], id_t[:])
                va = va_t[(b, j)]
                nc.vector.tensor_copy(
                    va[:].rearrange("p (g x) -> p g x", g=2)[:, :, 0:HD],
                    tp[:].rearrange("p (g x) -> p g x", g=2))

        def emit_proj_blocks(b, sts, yt):
            if NO_PROJ:
                return None
            it = sts[0] // 4
            if yt is None:
                yt = ysbp.tile([128, 4 * D], F16, tag="y")
            last = (sts[-1] + 1) % 4 == 0
            for stq in sts:
                ssl = slice(stq * 128, (stq + 1) * 128)
                c = stq % 4
                for eb in range(D // 512):
                    pp = ps_mm.tile([128, 512], F32, tag="mm", name="pp")
                    nc.tensor.matmul(
                        pp[:], ot2s[b][:, ssl],
                        pw_t[:, eb * 512:(eb + 1) * 512],
                        start=True, stop=True)
                    ysl = slice(c * D + eb * 512, c * D + (eb + 1) * 512)
                    if NO_YEV:
                        continue
                    if (2 * c + eb) % 4 < Y_TO_ACT:
                        nc.scalar.activation(
                            yt[:, ysl], pp[:],
                            mybir.ActivationFunctionType.Copy)
                    else:
                        nc.vector.tensor_copy(yt[:, ysl], pp[:])
            if last and not (SKIP_Y or NO_YEV):
                nc.sync.dma_start(y[b, it], yt[:])
            return yt

        def emit_att_stub(b, it):
            isl = slice(it * 512, (it + 1) * 512)
            nc.vector.tensor_copy(ot2s[b][:, isl], rpr_t[:, 0:512])
            emit_proj_blocks(b, range(4 * it, 4 * it + 4), None)

        pending_stages = []

        def pump(n=1):
            for _ in range(n):
                if pending_stages:
                    pending_stages.pop(0)()

        def flush_finish():
            pump(len(pending_stages))

        def emit_att_it(b, it):
            if NO_ATT:
                emit_att_stub(b, it)
                return
            # scores -> exp -> AV for one 512-wide query block, software-
            # pipelined two j-chunks deep so the in-order PE queue is not
            # gated on the ACT exp latency each chunk.  The normalize +
            # projection of each block is DEFERRED into the next block's
            # scores phase so the PE never stalls on the norm chain.
            qt, kt, _ = qkv[b]
            isl = slice(it * 512, (it + 1) * 512)
            jhi = (4 * it + 3) if mode == "causal" else (NJ - 1)
            otp = [ps_ot.tile([HD + 1, 512], F32, tag="ot", name=f"ot{_h}")
                   for _h in range(HPC)]

            def emit_scores(j):
                jsl = slice(j * 128, (j + 1) * 128)
                dc = max(0, (j - 4 * it) * 128) if mode == "causal" else 0
                dct = dc if SC_TRIM else 0
                if SC_SPLIT:
                    tiles = [ps_sc.tile([128, 512], F32, tag="sc",
                                        name=f"sc{_h}") for _h in range(HPC)]
                    views = [lambda lo, hi, t=t: t[:, lo:hi] for t in tiles]
                    whole = None
                else:
                    sc2 = ps_sc.tile([128, 2 * 512], F32, tag="sc",
                                     name="sc2")
                    views = [
                        lambda lo, hi, h=h: sc2[:, h * 512 + lo:h * 512 + hi]
                        for h in range(HPC)]
                    whole = sc2
                for h in range(HPC):
                    hsl = slice(h * HD, (h + 1) * HD)
                    nc.tensor.matmul(
                        views[h](dct, 512), kt[hsl, jsl],
                        qt[hsl, isl.start + dct:isl.stop], start=True,
                        stop=True, tile_position=(h * HD, 0))
                if maskT is not None:
                    mkt = mkp.tile([128, 512], F32, tag="mk")
                    nc.sync.dma_start(mkt[:], maskT[jsl, isl])
                    for h in range(HPC):
                        nc.vector.tensor_add(
                            views[h](0, 512), views[h](0, 512), mkt[:])
                return views, whole

            def emit_exp_av(j, sc):
                views, whole = sc
                dc = max(0, (j - 4 * it) * 128) if mode == "causal" else 0
                diag = mode == "causal" and j >= 4 * it
                pt2 = ptp.tile([128, 2 * 512], F16, tag="pt", name="pt2")
                if HALF_EXP:
                    # timing probe: exp only head 0; head 1 reads h0's P
                    nc.scalar.activation(
                        pt2[:, dc:512], views[0](dc, 512),
                        mybir.ActivationFunctionType.Exp, scale=SCALE)
                elif dc == 0 and whole is not None:
                    nc.scalar.activation(
                        pt2[:], whole[:],
                        mybir.ActivationFunctionType.Exp, scale=SCALE)
                else:
                    for h in range(HPC):
                        nc.scalar.activation(
                            pt2[:, h * 512 + dc:(h + 1) * 512],
                            views[h](dc, 512),
                            mybir.ActivationFunctionType.Exp, scale=SCALE)
                va = va_t[(b, j)]

                def ptb(h):
                    return 0 if HALF_EXP else h * 512

                eng = nc.gpsimd if TRIL_ENGINE == "pool" else nc.vector
                if diag and AV_SPLIT and j > 0:
                    # the diagonal 128x128 block needs the 0/1-tril multiply
                    # on the exp output; AV over the unmasked columns runs
                    # immediately, the masked-block AV trails off-path (the
                    # accumulation order into otp does not matter)
                    for h in range(HPC):
                        if dc + 128 < 512:
                            nc.tensor.matmul(
                                otp[h][:, dc + 128:512],
                                va[:, h * (HD + 1):(h + 1) * (HD + 1)],
                                pt2[:, ptb(h) + dc + 128:ptb(h) + 512],
                                start=(j == 0), stop=(j == jhi),
                                skip_group_check=True)
                        if not (HALF_EXP and h):
                            eng.tensor_mul(
                                pt2[:, ptb(h) + dc:ptb(h) + dc + 128],
                                pt2[:, ptb(h) + dc:ptb(h) + dc + 128],
                                tril_t[:])
                        nc.tensor.matmul(
                            otp[h][:, dc:dc + 128],
                            va[:, h * (HD + 1):(h + 1) * (HD + 1)],
                            pt2[:, ptb(h) + dc:ptb(h) + dc + 128],
                            start=(j == 0), stop=(j == jhi),
                            skip_group_check=True)
                else:
                    if diag:
                        for h in range(HPC):
                            if HALF_EXP and h:
                                continue
                            eng.tensor_mul(
                                pt2[:, ptb(h) + dc:ptb(h) + dc + 128],
                                pt2[:, ptb(h) + dc:ptb(h) + dc + 128],
                                tril_t[:])
                    for h in range(1 if HALF_AV else HPC):
                        nc.tensor.matmul(
                            otp[h][:, dc:512],
                            va[:, h * (HD + 1):(h + 1) * (HD + 1)],
                            pt2[:, ptb(h) + dc:ptb(h) + 512],
                            start=(j == 0), stop=(j == jhi),
                            skip_group_check=True)

            pend = [(0, emit_scores(0))]
            if jhi >= 1:
                pend.append((1, emit_scores(1)))
            pump()
            for j in range(2, jhi + 1):
                pend.append((j, emit_scores(j)))
                pump()
                jd, scd = pend.pop(0)
                emit_exp_av(jd, scd)
            flush_finish()
            for jd, scd in pend:
                emit_exp_av(jd, scd)

            # the normalize + projection chain is emitted as STAGES pumped
            # one scores-chunk apart during the NEXT block, so each hop's
            # input is complete before its (in-order) engine reaches it:
            # recs -> broadcasts -> multiplies -> proj(st01) -> proj(st23).
            # The OT_aug PSUM tiles are read directly by the rec + mul
            # stages (no SBUF eviction); the ps_ot ring (bufs=2) keeps the
            # next block in the other slot, and stage_mul retires this slot
            # well before block it+2 needs it.
            st = {}

            def stage_rec():
                st["rec"] = []
                for h in range(HPC):
                    op = otp[0] if HALF_AV else otp[h]
                    dn = smp.tile([1, 512], F32, tag="dn")
                    rec = smp.tile([1, 512], F32, tag="rec")
                    if not NO_NORM:
                        # ~18-bit 1/x in a single custom-DVE op; keeps the
                        # scalar engine free for the softmax exps.  The
                        # denom row goes through SBUF first — the custom op
                        # mis-reads PSUM sources.
                        nc.vector.tensor_copy(dn[:], op[HD:HD + 1, :])
                        nc.vector.reciprocal_approx_fast(rec[:], dn[:])
                    st["rec"].append(rec)

            def stage_bc():
                st["bc"] = []
                for h in range(HPC):
                    bc = smp.tile([HD, 512], F32, tag="bc")
                    if not NO_NORM:
                        nc.gpsimd.partition_broadcast(
                            bc[:], st["rec"][h][:])
                    st["bc"].append(bc)

            def stage_mul():
                for h in range(HPC):
                    op = otp[0] if HALF_AV else otp[h]
                    if NO_NORM:
                        src = rpr_t[0:HD, 0:512]
                    else:
                        src = st["bc"][h][:]
                    nc.vector.tensor_mul(
                        ot2s[b][h * HD:(h + 1) * HD, isl], op[0:HD, :],
                        src)

            def stage_proj01():
                st["yt"] = emit_proj_blocks(b, range(4 * it, 4 * it + 2),
                                            None)

            def stage_proj23():
                emit_proj_blocks(b, range(4 * it + 2, 4 * it + 4), st["yt"])

            pending_stages.extend(
                [stage_rec, stage_bc, stage_mul, stage_proj01, stage_proj23])

        def prologue():
            # one-time pipeline fill: batch 0's QKV ahead of the loop
            emit_loads(0)
            for g in range(3 * NI):
                emit_qkv_group(0, g)
            emit_va(0)

        def body(_iv=None):
            # Software-pipelined across iterations: each batch's (ACT-bound)
            # attention overlaps the OTHER batch's QKV projection groups on
            # the in-order PE queue; batch 0's QKV belongs to the NEXT
            # iteration (filled by the prologue for the first one).
            # interleave weights: att block `it` runs 4*(it+1) exp chunks, so
            # give later blocks proportionally more QKV filler groups
            gs, ge = [0, 0, 2, 6], [0, 2, 6, 12]
            emit_loads(1)
            for it in range(NI):
                emit_att_it(0, it)
                for g in range(gs[it], ge[it]):
                    emit_qkv_group(1, g)
            emit_va(1)
            emit_loads(0)
            for it in range(NI):
                emit_att_it(1, it)
                for g in range(gs[it], ge[it]):
                    emit_qkv_group(0, g)
            emit_va(0)
            flush_finish()

        return prologue, body

    nc._dbg = {"qkv": qkv, "ot2s": ot2s, "va": va_t, "xts": xts}
    prologue, body = make_body()
    prologue()
    if niter >= 1:
        for _ in range(niter):
            body()
    else:
        with tc.For_i(0, -niter, 1) as iv:
            body(iv)

    for c in reversed(ctxs):
        c.__exit__(None, None, None)


def _build(mode, niter=1):
    key = (mode, niter)
    if key in _BUILD_CACHE:
        return _BUILD_CACHE[key]
    nc = bacc.Bacc("TRN2", target_bir_lowering=False, debug=False,
                   num_devices=N_CORES)
    t = {}
    t["xt"] = nc.dram_tensor("xt", (B, 128, KC * S), F16, kind="ExternalInput")
    t["wqT"] = nc.dram_tensor("wqT", (D, DL), F16, kind="ExternalInput")
    t["wkT"] = nc.dram_tensor("wkT", (D, DL), F16, kind="ExternalInput")
    t["wvT"] = nc.dram_tensor("wvT", (D, DL), F16, kind="ExternalInput")
    t["pwT"] = nc.dram_tensor("pwT", (DL, D), F16, kind="ExternalInput")
    t["rpr2T"] = nc.dram_tensor("rpr2T", (128, S), F16, kind="ExternalInput")
    t["tril01"] = nc.dram_tensor("tril01", (128, 128), F16, kind="ExternalInput")
    t["ident"] = nc.dram_tensor("ident", (128, 128), F16, kind="ExternalInput")
    t["i2"] = nc.dram_tensor("i2", (128, 128), F16, kind="ExternalInput")
    t["onesc"] = nc.dram_tensor("onesc", (128, 1), F16, kind="ExternalInput")
    t["ones1"] = nc.dram_tensor("ones1", (1, HD), F16, kind="ExternalInput")
    if mode == "generic":
        t["maskT"] = nc.dram_tensor("maskT", (S, S), F32, kind="ExternalInput")
    t["y"] = nc.dram_tensor("y", (B, NI, 128, 4 * D), F16,
                            kind="ExternalOutput")

    with tile.TileContext(nc) as tc, \
            nc.allow_low_precision(reason="fp16 matmul operands"):
        _emit(nc, tc, t, mode, niter)
    nc.compile()
    _BUILD_CACHE[key] = (nc, t)
    return nc, t


def _prep_inputs(x, positions, causal_mask, wq, wk, wv, rpr, proj_w):
    """Host-side shard prep.  Returns (mode, per-core input maps)."""
    mask = np.asarray(causal_mask, np.float32).reshape(S, S)
    low = np.tril(np.ones((S, S), dtype=bool))
    if (mask[low] == 0.0).all() and (mask.any() and
                                     np.all(mask[~low] <= -1e6)):
        mode = "causal"
    elif not mask.any():
        mode = "zero"
    else:
        mode = "generic"

    # xt layout: (B, 128, KC*S): [b, p, k*S + s] = x[b, s, k*128 + p]
    xt = np.asarray(x, np.float32).transpose(0, 2, 1).reshape(B, KC, 128, S)
    xt = np.ascontiguousarray(xt.transpose(0, 2, 1, 3)).reshape(
        B, 128, KC * S).astype(np.float16)
    pos = np.asarray(positions).astype(np.int64)
    rpr_g = np.asarray(rpr, np.float32)[pos]  # (B, S, HD)
    rpr2 = np.ascontiguousarray(
        rpr_g.transpose(0, 2, 1)).reshape(B * HD, S).astype(np.float16)
    jj = np.arange(128)[:, None]
    ii = np.arange(128)[None, :]
    tril01 = (jj <= ii).astype(np.float16)
    ident = np.eye(128, dtype=np.float16)
    i2h = np.concatenate([np.eye(64), np.eye(64)], axis=1)
    i2 = np.concatenate([i2h, i2h], axis=0).astype(np.float16)
    maskT = np.ascontiguousarray(mask.T) if mode == "generic" else None

    wq = np.asarray(wq, np.float32)
    wk = np.asarray(wk, np.float32)
    wv = np.asarray(wv, np.float32)
    pw = np.asarray(proj_w, np.float32)

    in_maps = []
    for c in range(N_CORES):
        rs = slice(c * DL, (c + 1) * DL)
        m = {
            "xt": xt,
            "wqT": np.ascontiguousarray(wq[rs, :].T).astype(np.float16),
            "wkT": np.ascontiguousarray(wk[rs, :].T).astype(np.float16),
            "wvT": np.ascontiguousarray(wv[rs, :].T).astype(np.float16),
            "pwT": np.ascontiguousarray(pw[:, rs].T).astype(np.float16),
            "rpr2T": rpr2,
            "tril01": tril01,
            "ident": ident,
            "i2": i2,
            "onesc": np.ones((128, 1), np.float16),
            "ones1": np.ones((1, HD), np.float16),
        }
        if maskT is not None:
            m["maskT"] = maskT
        in_maps.append(m)
    return mode, in_maps


def kernel(x, positions, causal_mask, wq, wk, wv, rpr, proj_w, proj_b,
           _niter=1, **_ignored):
    mode, in_maps = _prep_inputs(x, positions, causal_mask, wq, wk, wv, rpr,
                                 proj_w)
    nc, _ = _build(mode, _niter)
    res = run_bass_kernel_spmd(nc, in_maps, core_ids=list(range(N_CORES)))
    out = np.zeros((B, S, D), dtype=np.float32)
    for r in res.results:
        # y layout: (B, NI, 128, 4*1024): [b, it, s, c*1024 + d]
        yr = r["y"].astype(np.float32).reshape(B, NI, 128, 4, D)
        out += yr.transpose(0, 1, 3, 2, 4).reshape(B, S, D)
    out += np.asarray(proj_b, np.float32)[None, None, :]
    return out

